# revision 5
# baseline (speedup 1.0000x reference)
"""Trainium2 Bass kernel for the DeepSeek-V4 indexer compressor (prefill).

Contract: kernel(**inputs) takes the FULL unsharded inputs (numpy) and
returns the FULL [1, 2048, 128] float32 output.

Strategy (8 NeuronCores, sequence-parallel):
  - Each core handles 1024 tokens = 256 compress blocks, plus a 4-token
    halo from the previous core (redundantly computed via a tiny side
    matmul) for the overlap transform.
  - On-device layout is channel-major: contraction dim (7168) lives on
    SBUF partitions, tokens on the free axis.  Host pre-transposes and
    bf16-casts x; weights for wkv/wgate are fused into one [7168, 512]
    matrix with channel order [kv_lo | sc_lo | kv_hi | sc_hi] and the
    per-phase position embedding (ape) folded in as 4 extra contraction
    rows fed by a 0/1 phase-indicator rhs.
  - Per 512-token half: matmul accumulates 57 k-chunks into 4 PSUM
    tiles; softmax over the 8 overlap rows is exp (ACT, fused with the
    PSUM->SBUF move) + strided quad-reduces (DVE); RMSNorm uses a
    ones-vector matmul + broadcast matmul; rotary is a pair-swap
    permutation matmul + 2 muls/add; the Hadamard rotation is a single
    128x128 matmul.  Output stays channel-major; host transposes back.
"""

import math
import os

import numpy as np
import ml_dtypes

import concourse.bass as bass
import concourse.bacc as bacc
import concourse.tile as tile
import concourse.mybir as mybir
from concourse.bass_utils import run_bass_kernel_spmd

BF16 = ml_dtypes.bfloat16
F32 = np.float32

# Problem dims (hardcoded per contract)
DIM = 7168
HD = 128
RATIO = 4
COFF = 2
SEQ = 8192
NB = SEQ // RATIO            # 2048 compressed blocks
NCORES = 8
TOK = SEQ // NCORES          # 1024 own tokens per core
NBC = TOK // RATIO           # 256 blocks per core
KC = DIM // 128              # 56 contraction chunks
G = 8                        # k-chunks per x/w DMA group
NG = KC // G                 # 7 groups
HALF = 512                   # tokens per matmul half
NH = TOK // HALF             # 2 halves
EPS = 1e-6
NEGB = -300.0                # exp(x - 300) == 0.0 in fp32 for the masked rows

_cache = {}


def _fwht_mat():
    """fwht(v) = M @ v for the reference's butterfly; returns lhsT = M.T.

    fwht applied to rows of I gives fwht(I)[r, k] = M[k, r] = M.T directly.
    """
    y = np.eye(HD, dtype=np.float64)
    d = HD
    for _ in range(int(math.log2(d))):
        y = y.reshape(y.shape[:-1] + (2, -1))
        a, b = y[..., 0, :], y[..., 1, :]
        y = np.concatenate([a + b, a - b], axis=-1)
    scale = np.float32(d) ** np.float32(-0.5)
    return (y * scale).astype(F32)


def _build_nc():
    nc = bacc.Bacc("TRN2", target_bir_lowering=False)
    f32 = mybir.dt.float32
    bf16 = mybir.dt.bfloat16

    # ---- DRAM I/O ----
    xp_d = nc.dram_tensor("xp", [NH * NG * 128, G * HALF], bf16, kind="ExternalInput")
    wp_d = nc.dram_tensor("wp", [NG * 128, G * HALF], bf16, kind="ExternalInput")
    hx_d = nc.dram_tensor("hx", [128, KC * RATIO], bf16, kind="ExternalInput")
    waug_d = nc.dram_tensor("waug", [RATIO, 512], bf16, kind="ExternalInput")
    xaug_d = nc.dram_tensor("xaug", [RATIO, 512], bf16, kind="ExternalInput")
    cdup_d = nc.dram_tensor("cdup", [128, NBC], f32, kind="ExternalInput")
    sdup_d = nc.dram_tensor("sdup", [128, NBC], f32, kind="ExternalInput")
    hbias_d = nc.dram_tensor("hbias", [128, 1], f32, kind="ExternalInput")
    normw_d = nc.dram_tensor("normw", [128, 1], f32, kind="ExternalInput")
    ones_d = nc.dram_tensor("onesk", [128, 1], f32, kind="ExternalInput")
    row1_d = nc.dram_tensor("row1", [1, 128], f32, kind="ExternalInput")
    pmat_d = nc.dram_tensor("pmat", [128, 128], f32, kind="ExternalInput")
    hmat_d = nc.dram_tensor("hmat", [128, 128], f32, kind="ExternalInput")
    out_d = nc.dram_tensor("out", [128, NBC], f32, kind="ExternalOutput")

    AX = mybir.AxisListType
    OP = mybir.AluOpType
    AF = mybir.ActivationFunctionType

    with tile.TileContext(nc) as tc:
        with (
            tc.tile_pool(name="wts", bufs=1) as wts,
            tc.tile_pool(name="csts", bufs=1) as csts,
            tc.tile_pool(name="xs", bufs=3) as xs,
            tc.tile_pool(name="epi", bufs=2) as epi,
            tc.tile_pool(name="ps", bufs=2, space="PSUM") as ps,
        ):
            # ---- weight + const loads ----
            wt = []
            for g in range(NG):
                wtg = wts.tile([128, G * HALF], bf16, name=f"wt{g}", tag=f"wt{g}")
                nc.scalar.dma_start(out=wtg, in_=wp_d[128 * g:128 * (g + 1), :])
                wt.append(wtg)

            hxsb = csts.tile([128, KC * RATIO], bf16, name="hxsb", tag="hxsb")
            nc.gpsimd.dma_start(out=hxsb, in_=hx_d[:, :])
            waug = csts.tile([RATIO, 512], bf16, name="waug", tag="waug")
            nc.gpsimd.dma_start(out=waug, in_=waug_d[:, :])
            xaug = csts.tile([RATIO, 512], bf16, name="xaug", tag="xaug")
            nc.gpsimd.dma_start(out=xaug, in_=xaug_d[:, :])
            cdup = csts.tile([128, NBC], f32, name="cdup", tag="cdup")
            nc.gpsimd.dma_start(out=cdup, in_=cdup_d[:, :])
            sdup = csts.tile([128, NBC], f32, name="sdup", tag="sdup")
            nc.gpsimd.dma_start(out=sdup, in_=sdup_d[:, :])
            hbias = csts.tile([128, 1], f32, name="hbias", tag="hbias")
            nc.gpsimd.dma_start(out=hbias, in_=hbias_d[:, :])
            normw = csts.tile([128, 1], f32, name="normw", tag="normw")
            nc.gpsimd.dma_start(out=normw, in_=normw_d[:, :])
            onesk = csts.tile([128, 1], f32, name="onesk", tag="onesk")
            nc.gpsimd.dma_start(out=onesk, in_=ones_d[:, :])
            row1 = csts.tile([1, 128], f32, name="row1", tag="row1")
            nc.gpsimd.dma_start(out=row1, in_=row1_d[:, :])
            pmat = csts.tile([128, 128], f32, name="pmat", tag="pmat")
            nc.gpsimd.dma_start(out=pmat, in_=pmat_d[:, :])
            hmat = csts.tile([128, 128], f32, name="hmat", tag="hmat")
            nc.gpsimd.dma_start(out=hmat, in_=hmat_d[:, :])

            epssb = csts.tile([1, 1], f32, name="epssb", tag="epssb")
            nc.vector.memset(epssb, EPS)

            halo_sb = csts.tile([128, 2 * RATIO], f32, name="halo_sb", tag="halo_sb")
            carry = csts.tile([128, 2 * RATIO], f32, name="carry", tag="carry")
            outsb = csts.tile([128, NBC], f32, name="outsb", tag="outsb")

            def main_matmuls(h, psums):
                """57-chunk accumulation over 4 m-tiles for 512 tokens."""
                for g in range(NG):
                    xg = xs.tile([128, G * HALF], bf16, name=f"xg{h}{g}", tag="xg")
                    row = (h * NG + g) * 128
                    nc.sync.dma_start(out=xg, in_=xp_d[row:row + 128, :])
                    for cc in range(G):
                        first = g == 0 and cc == 0
                        for m in range(4):
                            nc.tensor.matmul(
                                psums[m],
                                wt[g][:, cc * HALF + 128 * m:cc * HALF + 128 * (m + 1)],
                                xg[:, cc * HALF:(cc + 1) * HALF],
                                start=first,
                                stop=False,
                            )
                for m in range(4):
                    nc.tensor.matmul(
                        psums[m],
                        waug[:, 128 * m:128 * (m + 1)],
                        xaug[:, :],
                        start=False,
                        stop=True,
                    )

            def halo_matmuls():
                hp = ps.tile([128, HALF], mybir.dt.float32, name="hpsum", tag="kv1p")
                for m in range(2):  # kv_lo, sc_lo only
                    for c in range(KC):
                        g, cc = divmod(c, G)
                        nc.tensor.matmul(
                            hp[:, 4 * m:4 * (m + 1)],
                            wt[g][:, cc * HALF + 128 * m:cc * HALF + 128 * (m + 1)],
                            hxsb[:, 4 * c:4 * (c + 1)],
                            start=c == 0,
                            stop=False,
                        )
                    nc.tensor.matmul(
                        hp[:, 4 * m:4 * (m + 1)],
                        waug[:, 128 * m:128 * (m + 1)],
                        xaug[:, 0:RATIO],
                        start=False,
                        stop=True,
                    )
                nc.scalar.copy(out=halo_sb, in_=hp[:, 0:2 * RATIO])

            def epilogue(h, psums):
                kv1p, sc1p, kv2p, sc2p = psums
                prev = halo_sb if h == 0 else carry
                # E: [exp(sc1 window) | exp(sc2 window)], window shifted -4
                E = epi.tile([128, 2 * HALF], mybir.dt.float32, name=f"E{h}", tag="E")
                nc.scalar.activation(
                    E[:, 0:4], prev[:, RATIO:2 * RATIO], AF.Exp,
                    bias=(hbias[:, 0:1] if h == 0 else 0.0),
                )
                nc.scalar.activation(E[:, 4:HALF], sc1p[:, 0:HALF - 4], AF.Exp)
                nc.scalar.activation(E[:, HALF:2 * HALF], sc2p[:, :], AF.Exp)
                M = epi.tile([128, 2 * HALF], mybir.dt.float32, name=f"M{h}", tag="M")
                nc.vector.tensor_mul(M[:, 0:4], E[:, 0:4], prev[:, 0:RATIO])
                nc.vector.tensor_mul(M[:, 4:HALF], E[:, 4:HALF], kv1p[:, 0:HALF - 4])
                nc.vector.tensor_mul(M[:, HALF:2 * HALF], E[:, HALF:2 * HALF], kv2p[:, :])
                if h == 0:
                    # carry the last 4 token columns of kv_lo/sc_lo into half 1
                    nc.scalar.copy(out=carry[:, 0:RATIO], in_=kv1p[:, HALF - 4:HALF])
                    nc.scalar.copy(out=carry[:, RATIO:2 * RATIO], in_=sc1p[:, HALF - 4:HALF])

                nloc = HALF // RATIO  # 128 blocks in this half
                Z = epi.tile([128, nloc], mybir.dt.float32, name=f"Z{h}", tag="Z")
                nc.vector.tensor_reduce(
                    Z, E.rearrange("p (s n q) -> p n s q", s=2, q=RATIO),
                    axis=AX.XY, op=OP.add,
                )
                A = epi.tile([128, nloc], mybir.dt.float32, name=f"A{h}", tag="A")
                nc.vector.tensor_reduce(
                    A, M.rearrange("p (s n q) -> p n s q", s=2, q=RATIO),
                    axis=AX.XY, op=OP.add,
                )
                Zr = epi.tile([128, nloc], mybir.dt.float32, name=f"Zr{h}", tag="Zr")
                nc.vector.reciprocal(Zr, Z)
                comp = epi.tile([128, nloc], mybir.dt.float32, name=f"comp{h}", tag="comp")
                nc.vector.tensor_mul(comp, A, Zr)
                sq = epi.tile([128, nloc], mybir.dt.float32, name=f"sq{h}", tag="sq")
                nc.scalar.square(sq, comp)

                misc = ps.tile([128, HALF], mybir.dt.float32, name=f"misc{h}", tag="kv2p")
                # mean(comp^2) over channels: ones(1/128) matmul -> [1, nloc]
                nc.tensor.matmul(misc[0:1, 384:384 + nloc], onesk[:, :], sq[:, :],
                                 start=True, stop=True)
                sd = epi.tile([1, nloc], mybir.dt.float32, name=f"sd{h}", tag="sd")
                nc.scalar.activation(sd, misc[0:1, 384:384 + nloc], AF.Sqrt,
                                     bias=epssb[0:1, 0:1])
                rs = epi.tile([1, nloc], mybir.dt.float32, name=f"rs{h}", tag="rs")
                nc.vector.reciprocal(rs, sd)
                # broadcast rs across partitions: [1,128] ones lhsT
                nc.tensor.matmul(misc[:, 0:nloc], row1[:, :], rs[:, :],
                                 start=True, stop=True)
                compn = epi.tile([128, nloc], mybir.dt.float32, name=f"compn{h}", tag="compn")
                nc.vector.scalar_tensor_tensor(
                    out=compn, in0=comp, scalar=normw[:, 0:1], in1=misc[:, 0:nloc],
                    op0=OP.mult, op1=OP.mult,
                )
                # rotary: pair-swap matmul, then compn*cdup + P(compn)*sdup
                nc.tensor.matmul(misc[:, 128:128 + nloc], pmat[:, :], compn[:, :],
                                 start=True, stop=True)
                t1 = epi.tile([128, nloc], mybir.dt.float32, name=f"t1{h}", tag="t1")
                nc.vector.tensor_mul(t1, misc[:, 128:128 + nloc],
                                     sdup[:, h * nloc:(h + 1) * nloc])
                t2 = epi.tile([128, nloc], mybir.dt.float32, name=f"t2{h}", tag="t2")
                nc.vector.tensor_mul(t2, compn, cdup[:, h * nloc:(h + 1) * nloc])
                rot = epi.tile([128, nloc], mybir.dt.float32, name=f"rot{h}", tag="rot")
                nc.vector.tensor_add(rot, t1, t2)
                # Hadamard rotation
                nc.tensor.matmul(misc[:, 256:256 + nloc], hmat[:, :], rot[:, :],
                                 start=True, stop=True)
                nc.scalar.copy(out=outsb[:, h * nloc:(h + 1) * nloc],
                               in_=misc[:, 256:256 + nloc])

            for h in range(NH):
                psums = []
                for m, nm in enumerate(("kv1p", "sc1p", "kv2p", "sc2p")):
                    p = ps.tile([128, HALF], mybir.dt.float32, name=f"{nm}_{h}", tag=nm)
                    psums.append(p)
                main_matmuls(h, psums)
                if h == 0:
                    halo_matmuls()
                epilogue(h, psums)

            nc.sync.dma_start(out=out_d[:, :], in_=outsb)

    nc.finalize()
    return nc


def _prep_inputs(x, ape, wkv_w, wgate_w, norm_w, cos, sin):
    """Host-side packing of per-core input maps."""
    x = np.asarray(x, dtype=F32)[0]          # [SEQ, DIM]
    ape = np.asarray(ape, dtype=F32)         # [RATIO, 256]
    wkv_w = np.asarray(wkv_w, dtype=F32)     # [256, DIM]
    wgate_w = np.asarray(wgate_w, dtype=F32)
    norm_w = np.asarray(norm_w, dtype=F32)   # [HD]
    cos = np.asarray(cos, dtype=F32)         # [NB, 32]
    sin = np.asarray(sin, dtype=F32)

    xb = x.astype(BF16)

    # fused weights, channel order [kv_lo | sc_lo | kv_hi | sc_hi]
    w_comb = np.concatenate(
        [wkv_w[0:128], wgate_w[0:128], wkv_w[128:256], wgate_w[128:256]], axis=0
    )  # [512, DIM]
    wp = (
        w_comb.T.reshape(NG, G, 128, 512)
        .transpose(0, 2, 1, 3)
        .reshape(NG * 128, G * HALF)
        .astype(BF16)
    )
    wp = np.ascontiguousarray(wp)

    # ape K-augmentation weights [4, 512]
    waug = np.zeros((RATIO, 512), dtype=F32)
    waug[:, 0:128] = ape[:, 0:128]
    waug[:, 256:384] = ape[:, 128:256]
    waug = waug.astype(BF16)
    # phase indicator rhs [4, 512]
    xaug = np.zeros((RATIO, 512), dtype=F32)
    for p in range(RATIO):
        xaug[p, p::RATIO] = 1.0
    xaug = xaug.astype(BF16)

    normw = np.ascontiguousarray(norm_w.reshape(128, 1))
    onesk = np.full((128, 1), 1.0 / HD, dtype=F32)
    row1 = np.ones((1, 128), dtype=F32)

    pmat = np.zeros((128, 128), dtype=F32)
    for i in range(RD2 := 32):
        pmat[2 * i, 2 * i + 1] = 1.0
        pmat[2 * i + 1, 2 * i] = 1.0
    hmat = _fwht_mat()

    in_maps = []
    for c in range(NCORES):
        t0 = c * TOK
        seg = xb[t0:t0 + TOK]                       # [1024, DIM]
        arr = np.ascontiguousarray(seg.T)           # [DIM, 1024]
        arr = arr.reshape(KC, 128, NH, HALF)        # [c, p, h, t]
        xp = (
            arr.transpose(2, 0, 1, 3)               # [h, c, p, t]
            .reshape(NH, NG, G, 128, HALF)
            .transpose(0, 1, 3, 2, 4)               # [h, g, p, cc, t]
            .reshape(NH * NG * 128, G * HALF)
        )
        xp = np.ascontiguousarray(xp)

        if c == 0:
            halo = np.zeros((RATIO, DIM), dtype=BF16)
        else:
            halo = xb[t0 - RATIO:t0]
        hx = np.ascontiguousarray(
            halo.T.reshape(KC, 128, RATIO).transpose(1, 0, 2).reshape(128, KC * RATIO)
        )

        b0 = c * NBC
        cs = cos[b0:b0 + NBC]                       # [NBC, 32]
        ss = sin[b0:b0 + NBC]
        cdup = np.ones((128, NBC), dtype=F32)
        sdup = np.zeros((128, NBC), dtype=F32)
        cdup[0:64:2] = cs.T[np.arange(32)]
        cdup[1:64:2] = cs.T[np.arange(32)]
        sdup[0:64:2] = -ss.T[np.arange(32)]
        sdup[1:64:2] = ss.T[np.arange(32)]

        hbias = np.zeros((128, 1), dtype=F32)
        if c == 0:
            hbias[:] = NEGB

        in_maps.append(dict(
            xp=xp, wp=wp, hx=hx, waug=waug, xaug=xaug,
            cdup=np.ascontiguousarray(cdup), sdup=np.ascontiguousarray(sdup),
            hbias=hbias, normw=normw, onesk=onesk, row1=row1,
            pmat=pmat, hmat=hmat,
        ))
    return in_maps


LAST_RESULTS = None


def kernel(x, ape, wkv_w, wgate_w, norm_w, cos, sin, start_pos=0,
           compress_state=None, **_unused):
    global LAST_RESULTS
    in_maps = _prep_inputs(x, ape, wkv_w, wgate_w, norm_w, cos, sin)
    if "nc" not in _cache:
        _cache["nc"] = _build_nc()
    nc = _cache["nc"]
    trace = bool(int(os.environ.get("KERNEL_TRACE", "0") or 0))
    tdir = os.environ.get("KERNEL_TRACE_DIR") or None
    res = run_bass_kernel_spmd(
        nc, in_maps, core_ids=list(range(NCORES)),
        trace=trace,
        trace_cores=[0] if trace else None,
        tmpdir=tdir,
    )
    LAST_RESULTS = res
    out = np.empty((1, NB, HD), dtype=F32)
    for c in range(NCORES):
        out[0, c * NBC:(c + 1) * NBC, :] = res.results[c]["out"].T
    return out


# revision 9
# speedup vs baseline: 1.0116x; 1.0116x over previous
"""Trainium2 Bass kernel for the DeepSeek-V4 indexer compressor (prefill).

Contract: kernel(**inputs) takes the FULL unsharded inputs (numpy) and
returns the FULL [1, 2048, 128] float32 output.

Strategy (8 NeuronCores, sequence-parallel):
  - Each core handles 1024 tokens = 256 compress blocks, plus a 4-token
    halo from the previous core (redundantly computed via tiny N=4
    matmuls interleaved into the main stream) for the overlap transform.
  - On-device layout is channel-major: contraction dim (7168) lives on
    SBUF partitions, tokens on the free axis.  Host pre-transposes and
    bf16-casts x; wkv/wgate are fused into one [7168, 512] matrix with
    channel order [kv_lo | sc_lo | kv_hi | sc_hi] and the per-phase
    position embedding (ape) folded in as 4 extra contraction rows fed
    by a 0/1 phase-indicator rhs.
  - Per 512-token half: matmuls accumulate 57 k-chunks into 4 PSUM
    tiles; the epilogue (softmax over the 8 overlap rows, RMSNorm,
    rotary, Hadamard) runs in two pipelined 64-block sub-chunks:
    exp on ACT straight out of PSUM, strided quad-reduces on DVE,
    RMSNorm via ones-matmul + broadcast-matmul, rotary via a pair-swap
    permutation matmul, FWHT as one 128x128 matmul.  Output stays
    channel-major; host transposes back.
"""

import math
import os

import numpy as np
import ml_dtypes

import concourse.bass as bass
import concourse.bacc as bacc
import concourse.tile as tile
import concourse.mybir as mybir
from concourse.bass_utils import run_bass_kernel_spmd

BF16 = ml_dtypes.bfloat16
F32 = np.float32

# Problem dims (hardcoded per contract)
DIM = 7168
HD = 128
RATIO = 4
COFF = 2
SEQ = 8192
NB = SEQ // RATIO            # 2048 compressed blocks
NCORES = 8
TOK = SEQ // NCORES          # 1024 own tokens per core
NBC = TOK // RATIO           # 256 blocks per core
KC = DIM // 128              # 56 contraction chunks
G = 8                        # k-chunks per x/w DMA group
NG = KC // G                 # 7 groups
HALF = 512                   # tokens per matmul half
NH = TOK // HALF             # 2 halves
NSUB = 2                     # epilogue sub-chunks per half
NLOC = HALF // RATIO // NSUB  # 64 blocks per sub-chunk
EPS = 1e-6
NEGB = -300.0                # exp(x - 300) == 0.0 in fp32 for the masked rows

# f32 const pack column layout
C_CD = 0            # cdup [128, 256]
C_SD = 256          # sdup [128, 256]
C_PM = 512          # pmat [128, 128]
C_HM = 640          # hmat [128, 128]
C_NW = 768          # norm_w [128, 1]
C_OK = 769          # ones/HD [128, 1]
C_HB = 770          # halo bias [128, 1]
C_R1 = 771          # all-ones block [128, 128] (row 0 used as [1,128])
C_EPS = 899         # eps [1, 1] (at [0:1, 899])
C_TOT = 900

_cache = {}


def _fwht_mat():
    """fwht(v) = M @ v for the reference's butterfly; fwht(I) = M.T which
    is exactly the lhsT the tensor engine wants."""
    y = np.eye(HD, dtype=np.float64)
    d = HD
    for _ in range(int(math.log2(d))):
        y = y.reshape(y.shape[:-1] + (2, -1))
        a, b = y[..., 0, :], y[..., 1, :]
        y = np.concatenate([a + b, a - b], axis=-1)
    scale = np.float32(d) ** np.float32(-0.5)
    return (y * scale).astype(F32)


def _build_nc():
    nc = bacc.Bacc("TRN2", target_bir_lowering=False)
    f32 = mybir.dt.float32
    bf16 = mybir.dt.bfloat16

    xp_d = nc.dram_tensor("xp", [NH * NG * 128, G * HALF], bf16, kind="ExternalInput")
    wp_d = nc.dram_tensor("wp", [NG * 128, G * HALF], bf16, kind="ExternalInput")
    hx_d = nc.dram_tensor("hx", [128, KC * RATIO], bf16, kind="ExternalInput")
    aug_d = nc.dram_tensor("aug", [RATIO, 1024], bf16, kind="ExternalInput")
    cpk_d = nc.dram_tensor("cpk", [128, C_TOT], f32, kind="ExternalInput")
    out_d = nc.dram_tensor("out", [128, NBC], f32, kind="ExternalOutput")

    AX = mybir.AxisListType
    OP = mybir.AluOpType
    AF = mybir.ActivationFunctionType

    with tile.TileContext(nc) as tc:
        with (
            tc.tile_pool(name="wts", bufs=1) as wts,
            tc.tile_pool(name="csts", bufs=1) as csts,
            tc.tile_pool(name="xs", bufs=4) as xs,
            tc.tile_pool(name="epi", bufs=2) as epi,
            tc.tile_pool(name="ps", bufs=2, space="PSUM") as ps,
            tc.tile_pool(name="mps", bufs=1, space="PSUM") as mps,
        ):
            # one permanent PSUM bank for the small matmul outputs + halo:
            # [0:64 bcast][128:192 perm][256:320 fwht][384:448 varsum][448:456 halo]
            misc = mps.tile([128, HALF], mybir.dt.float32, name="misc", tag="misc")
            # ---- const loads (SWDGE, off the x/w queue) ----
            hxsb = csts.tile([128, KC * RATIO], bf16, name="hxsb", tag="hxsb")
            nc.gpsimd.dma_start(out=hxsb, in_=hx_d[:, :])
            aug = csts.tile([RATIO, 1024], bf16, name="aug", tag="aug")
            nc.gpsimd.dma_start(out=aug, in_=aug_d[:, :])
            cpk = csts.tile([128, C_TOT], f32, name="cpk", tag="cpk")
            nc.gpsimd.dma_start(out=cpk, in_=cpk_d[:, :])
            waug = aug[:, 0:512]
            xaug = aug[:, 512:1024]
            cdup = cpk[:, C_CD:C_CD + NBC]
            sdup = cpk[:, C_SD:C_SD + NBC]
            pmat = cpk[:, C_PM:C_PM + 128]
            hmat = cpk[:, C_HM:C_HM + 128]
            normw = cpk[:, C_NW:C_NW + 1]
            onesk = cpk[:, C_OK:C_OK + 1]
            hbias = cpk[:, C_HB:C_HB + 1]
            row1 = cpk[0:1, C_R1:C_R1 + 128]
            epsap = cpk[0:1, C_EPS:C_EPS + 1]

            halo_sb = csts.tile([128, 2 * RATIO], f32, name="halo_sb", tag="halo_sb")
            carry = csts.tile([128, 2 * RATIO], f32, name="carry", tag="carry")
            outsb = csts.tile([128, NBC], f32, name="outsb", tag="outsb")

            # ---- interleaved weight/x loads on the sync HWDGE queue ----
            wt = []
            xg_tiles = {}
            for g in range(NG):
                wtg = wts.tile([128, G * HALF], bf16, name=f"wt{g}", tag=f"wt{g}")
                nc.sync.dma_start(out=wtg, in_=wp_d[128 * g:128 * (g + 1), :])
                wt.append(wtg)
                xg = xs.tile([128, G * HALF], bf16, name=f"xg0{g}", tag="xg")
                nc.sync.dma_start(out=xg, in_=xp_d[128 * g:128 * (g + 1), :])
                xg_tiles[(0, g)] = xg

            def main_matmuls(h, psums, halo_psum):
                for g in range(NG):
                    if h == 0:
                        xg = xg_tiles.pop((0, g))
                    else:
                        xg = xs.tile([128, G * HALF], bf16, name=f"xg1{g}", tag="xg")
                        row = (h * NG + g) * 128
                        nc.sync.dma_start(out=xg, in_=xp_d[row:row + 128, :])
                    for cc in range(G):
                        first = g == 0 and cc == 0
                        for m in range(4):
                            nc.tensor.matmul(
                                psums[m],
                                wt[g][:, cc * HALF + 128 * m:cc * HALF + 128 * (m + 1)],
                                xg[:, cc * HALF:(cc + 1) * HALF],
                                start=first,
                                stop=False,
                            )
                    if h == 0:
                        # halo mini-matmuls ride along with the k-group whose
                        # weights are resident; keeps PE activity dense.
                        for cc in range(G):
                            c = g * G + cc
                            for m in range(2):
                                nc.tensor.matmul(
                                    halo_psum[:, 4 * m:4 * (m + 1)],
                                    wt[g][:, cc * HALF + 128 * m:cc * HALF + 128 * (m + 1)],
                                    hxsb[:, 4 * c:4 * (c + 1)],
                                    start=c == 0,
                                    stop=False,
                                )
                for m in range(4):
                    nc.tensor.matmul(
                        psums[m],
                        waug[:, 128 * m:128 * (m + 1)],
                        xaug[:, :],
                        start=False,
                        stop=True,
                    )
                if h == 0:
                    for m in range(2):
                        nc.tensor.matmul(
                            halo_psum[:, 4 * m:4 * (m + 1)],
                            waug[:, 128 * m:128 * (m + 1)],
                            xaug[:, 0:RATIO],
                            start=False,
                            stop=True,
                        )
                    nc.scalar.copy(out=halo_sb, in_=halo_psum[:, 0:2 * RATIO])

            def epilogue_sub(h, s, psums, E, M):
                """Softmax+RMS+rotary+FWHT for blocks [64s, 64s+64) of half h."""
                kv1p, sc1p, kv2p, sc2p = psums
                prev = halo_sb if h == 0 else carry
                c0 = 256 * s                      # E/M column base (E1 part)
                nq = NLOC * RATIO                 # 256 token-cols per sub
                if s == 0:
                    nc.scalar.activation(
                        E[:, 0:4], prev[:, RATIO:2 * RATIO], AF.Exp,
                        bias=(hbias if h == 0 else 0.0),
                    )
                    nc.scalar.activation(E[:, 4:nq], sc1p[:, 0:nq - 4], AF.Exp)
                    nc.vector.tensor_mul(M[:, 0:4], E[:, 0:4], prev[:, 0:RATIO])
                    nc.vector.tensor_mul(M[:, 4:nq], E[:, 4:nq], kv1p[:, 0:nq - 4])
                else:
                    nc.scalar.activation(E[:, c0:c0 + nq], sc1p[:, c0 - 4:c0 + nq - 4],
                                         AF.Exp)
                    nc.vector.tensor_mul(M[:, c0:c0 + nq], E[:, c0:c0 + nq],
                                         kv1p[:, c0 - 4:c0 + nq - 4])
                nc.scalar.activation(E[:, 512 + c0:512 + c0 + nq],
                                     sc2p[:, c0:c0 + nq], AF.Exp)
                nc.vector.tensor_mul(M[:, 512 + c0:512 + c0 + nq],
                                     E[:, 512 + c0:512 + c0 + nq],
                                     kv2p[:, c0:c0 + nq])

                Er = E.rearrange("p (t n q) -> p t n q", t=2, q=RATIO)
                Mr = M.rearrange("p (t n q) -> p t n q", t=2, q=RATIO)
                n0 = NLOC * s
                Z = epi.tile([128, NLOC], mybir.dt.float32, name=f"Z{h}{s}", tag="Z")
                nc.vector.tensor_reduce(
                    Z, Er[:, :, n0:n0 + NLOC, :].rearrange("p t n q -> p n t q"),
                    axis=AX.XY, op=OP.add)
                A = epi.tile([128, NLOC], mybir.dt.float32, name=f"A{h}{s}", tag="A")
                nc.vector.tensor_reduce(
                    A, Mr[:, :, n0:n0 + NLOC, :].rearrange("p t n q -> p n t q"),
                    axis=AX.XY, op=OP.add)
                Zr = epi.tile([128, NLOC], mybir.dt.float32, name=f"Zr{h}{s}", tag="Zr")
                nc.vector.reciprocal(Zr, Z)
                comp = epi.tile([128, NLOC], mybir.dt.float32, name=f"cp{h}{s}", tag="cp")
                nc.vector.tensor_mul(comp, A, Zr)
                sq = epi.tile([128, NLOC], mybir.dt.float32, name=f"sq{h}{s}", tag="sq")
                nc.scalar.square(sq, comp)

                nc.tensor.matmul(misc[0:1, 384:384 + NLOC], onesk, sq[:, :],
                                 start=True, stop=True)
                sd = epi.tile([1, NLOC], mybir.dt.float32, name=f"sd{h}{s}", tag="sd")
                nc.scalar.activation(sd, misc[0:1, 384:384 + NLOC], AF.Sqrt,
                                     bias=epsap)
                rs = epi.tile([1, NLOC], mybir.dt.float32, name=f"rs{h}{s}", tag="rs")
                nc.vector.reciprocal(rs, sd)
                nc.tensor.matmul(misc[:, 0:NLOC], row1, rs[:, :],
                                 start=True, stop=True)
                compn = epi.tile([128, NLOC], mybir.dt.float32,
                                 name=f"cn{h}{s}", tag="cn")
                nc.vector.scalar_tensor_tensor(
                    out=compn, in0=comp, scalar=normw, in1=misc[:, 0:NLOC],
                    op0=OP.mult, op1=OP.mult)
                nc.tensor.matmul(misc[:, 128:128 + NLOC], pmat, compn[:, :],
                                 start=True, stop=True)
                b0 = h * HALF // RATIO + n0       # global block base in this core
                t1 = epi.tile([128, NLOC], mybir.dt.float32, name=f"t1{h}{s}", tag="t1")
                nc.vector.tensor_mul(t1, misc[:, 128:128 + NLOC],
                                     sdup[:, b0:b0 + NLOC])
                t2 = epi.tile([128, NLOC], mybir.dt.float32, name=f"t2{h}{s}", tag="t2")
                nc.vector.tensor_mul(t2, compn, cdup[:, b0:b0 + NLOC])
                rot = epi.tile([128, NLOC], mybir.dt.float32, name=f"rt{h}{s}", tag="rt")
                nc.vector.tensor_add(rot, t1, t2)
                nc.tensor.matmul(misc[:, 256:256 + NLOC], hmat, rot[:, :],
                                 start=True, stop=True)
                nc.scalar.copy(out=outsb[:, b0:b0 + NLOC],
                               in_=misc[:, 256:256 + NLOC])

            for h in range(NH):
                psums = []
                for m, nm in enumerate(("kv1p", "sc1p", "kv2p", "sc2p")):
                    p = ps.tile([128, HALF], mybir.dt.float32, name=f"{nm}_{h}",
                                tag=nm, bufs=1 if nm == "sc2p" else 2)
                    psums.append(p)
                halo_psum = misc[:, 448:456] if h == 0 else None
                main_matmuls(h, psums, halo_psum)
                if h == 0:
                    # carry last 4 token-cols of kv_lo/sc_lo into half 1
                    nc.scalar.copy(out=carry[:, 0:RATIO], in_=psums[0][:, HALF - 4:HALF])
                    nc.scalar.copy(out=carry[:, RATIO:2 * RATIO],
                                   in_=psums[1][:, HALF - 4:HALF])
                E = epi.tile([128, 2 * HALF], mybir.dt.float32, name=f"E{h}", tag="E")
                M = epi.tile([128, 2 * HALF], mybir.dt.float32, name=f"M{h}", tag="M")
                for s in range(NSUB):
                    epilogue_sub(h, s, psums, E, M)

            nc.sync.dma_start(out=out_d[:, :], in_=outsb)

    nc.finalize()
    return nc


def _prep_inputs(x, ape, wkv_w, wgate_w, norm_w, cos, sin):
    """Host-side packing of per-core input maps."""
    x = np.asarray(x, dtype=F32)[0]          # [SEQ, DIM]
    ape = np.asarray(ape, dtype=F32)         # [RATIO, 256]
    wkv_w = np.asarray(wkv_w, dtype=F32)     # [256, DIM]
    wgate_w = np.asarray(wgate_w, dtype=F32)
    norm_w = np.asarray(norm_w, dtype=F32)   # [HD]
    cos = np.asarray(cos, dtype=F32)         # [NB, 32]
    sin = np.asarray(sin, dtype=F32)

    xb = x.astype(BF16)

    w_comb = np.concatenate(
        [wkv_w[0:128], wgate_w[0:128], wkv_w[128:256], wgate_w[128:256]], axis=0
    )  # [512, DIM]
    wp = (
        w_comb.T.reshape(NG, G, 128, 512)
        .transpose(0, 2, 1, 3)
        .reshape(NG * 128, G * HALF)
        .astype(BF16)
    )
    wp = np.ascontiguousarray(wp)

    # aug: [waug | xaug] as one [4, 1024] bf16 tensor
    aug = np.zeros((RATIO, 1024), dtype=F32)
    aug[:, 0:128] = ape[:, 0:128]
    aug[:, 256:384] = ape[:, 128:256]
    for p in range(RATIO):
        aug[p, 512 + p::RATIO] = 1.0
    aug = aug.astype(BF16)

    pmat = np.zeros((128, 128), dtype=F32)
    for i in range(32):
        pmat[2 * i, 2 * i + 1] = 1.0
        pmat[2 * i + 1, 2 * i] = 1.0
    hmat = _fwht_mat()

    in_maps = []
    for c in range(NCORES):
        t0 = c * TOK
        seg = xb[t0:t0 + TOK]                       # [1024, DIM]
        arr = np.ascontiguousarray(seg.T)           # [DIM, 1024]
        arr = arr.reshape(KC, 128, NH, HALF)        # [c, p, h, t]
        xp = (
            arr.transpose(2, 0, 1, 3)               # [h, c, p, t]
            .reshape(NH, NG, G, 128, HALF)
            .transpose(0, 1, 3, 2, 4)               # [h, g, p, cc, t]
            .reshape(NH * NG * 128, G * HALF)
        )
        xp = np.ascontiguousarray(xp)

        if c == 0:
            halo = np.zeros((RATIO, DIM), dtype=BF16)
        else:
            halo = xb[t0 - RATIO:t0]
        hx = np.ascontiguousarray(
            halo.T.reshape(KC, 128, RATIO).transpose(1, 0, 2).reshape(128, KC * RATIO)
        )

        b0 = c * NBC
        cs = cos[b0:b0 + NBC]                       # [NBC, 32]
        ss = sin[b0:b0 + NBC]
        cpk = np.zeros((128, C_TOT), dtype=F32)
        cd = np.ones((128, NBC), dtype=F32)
        sd = np.zeros((128, NBC), dtype=F32)
        cd[0:64:2] = cs.T
        cd[1:64:2] = cs.T
        sd[0:64:2] = -ss.T
        sd[1:64:2] = ss.T
        cpk[:, C_CD:C_CD + NBC] = cd
        cpk[:, C_SD:C_SD + NBC] = sd
        cpk[:, C_PM:C_PM + 128] = pmat
        cpk[:, C_HM:C_HM + 128] = hmat
        cpk[:, C_NW] = norm_w
        cpk[:, C_OK] = 1.0 / HD
        cpk[:, C_HB] = NEGB if c == 0 else 0.0
        cpk[:, C_R1:C_R1 + 128] = 1.0
        cpk[0, C_EPS] = EPS

        in_maps.append(dict(xp=xp, wp=wp, hx=hx, aug=aug,
                            cpk=np.ascontiguousarray(cpk)))
    return in_maps


LAST_RESULTS = None


def kernel(x, ape, wkv_w, wgate_w, norm_w, cos, sin, start_pos=0,
           compress_state=None, **_unused):
    global LAST_RESULTS
    in_maps = _prep_inputs(x, ape, wkv_w, wgate_w, norm_w, cos, sin)
    if "nc" not in _cache:
        _cache["nc"] = _build_nc()
    nc = _cache["nc"]
    trace = bool(int(os.environ.get("KERNEL_TRACE", "0") or 0))
    tdir = os.environ.get("KERNEL_TRACE_DIR") or None
    res = run_bass_kernel_spmd(
        nc, in_maps, core_ids=list(range(NCORES)),
        trace=trace,
        trace_cores=[0] if trace else None,
        tmpdir=tdir,
    )
    LAST_RESULTS = res
    out = np.empty((1, NB, HD), dtype=F32)
    for c in range(NCORES):
        out[0, c * NBC:(c + 1) * NBC, :] = res.results[c]["out"].T
    return out


# revision 15
# speedup vs baseline: 1.0242x; 1.0124x over previous
"""Trainium2 Bass kernel for the DeepSeek-V4 indexer compressor (prefill).

Contract: kernel(**inputs) takes the FULL unsharded inputs (numpy) and
returns the FULL [1, 2048, 128] float32 output.

Strategy (8 NeuronCores, sequence-parallel):
  - Each core handles 1024 tokens = 256 compress blocks, plus a 4-token
    halo from the previous core (redundantly computed via tiny N=4
    matmuls interleaved into the main stream) for the overlap transform.
  - On-device layout is channel-major: contraction dim (7168) lives on
    SBUF partitions, tokens on the free axis.  Host pre-transposes and
    bf16-casts x; wkv/wgate are fused into one [7168, 512] matrix with
    channel order [kv_lo | sc_lo | kv_hi | sc_hi] and the per-phase
    position embedding (ape) folded in as 4 extra contraction rows fed
    by a 0/1 phase-indicator rhs.
  - Per 512-token half: matmuls accumulate 57 k-chunks into 4 PSUM
    tiles; the epilogue (softmax over the 8 overlap rows, RMSNorm,
    rotary, Hadamard) runs in two pipelined 64-block sub-chunks:
    exp on ACT straight out of PSUM, strided quad-reduces on DVE,
    RMSNorm via ones-matmul + broadcast-matmul, rotary via a pair-swap
    permutation matmul, FWHT as one 128x128 matmul.  Output stays
    channel-major; host transposes back.
"""

import math
import os

import numpy as np
import ml_dtypes

import concourse.bass as bass
import concourse.bacc as bacc
import concourse.tile as tile
import concourse.mybir as mybir
from concourse.bass_utils import run_bass_kernel_spmd

BF16 = ml_dtypes.bfloat16
F32 = np.float32

# Problem dims (hardcoded per contract)
DIM = 7168
HD = 128
RATIO = 4
COFF = 2
SEQ = 8192
NB = SEQ // RATIO            # 2048 compressed blocks
NCORES = 8
TOK = SEQ // NCORES          # 1024 own tokens per core
NBC = TOK // RATIO           # 256 blocks per core
KC = DIM // 128              # 56 contraction chunks
G = 8                        # k-chunks per x/w DMA group
NG = KC // G                 # 7 groups
HALF = 512                   # tokens per matmul half
NH = TOK // HALF             # 2 halves
NSUB = 2                     # epilogue sub-chunks per half
NLOC = HALF // RATIO // NSUB  # 64 blocks per sub-chunk
EPS = 1e-6
NEGB = -300.0                # exp(x - 300) == 0.0 in fp32 for the masked rows

# f32 const pack column layout
C_CD = 0            # cdup [128, 256]
C_SD = 256          # sdup [128, 256]
C_PM = 512          # pmat [128, 128]
C_HM = 640          # hmat [128, 128]
C_NW = 768          # norm_w [128, 1]
C_OK = 769          # ones/HD [128, 1]
C_HB = 770          # halo bias [128, 1]
C_R1 = 771          # all-ones block [128, 128] (row 0 used as [1,128])
C_EPS = 899         # eps [1, 1] (at [0:1, 899])
C_TOT = 900

_cache = {}


def _fwht_mat():
    """fwht(v) = M @ v for the reference's butterfly; fwht(I) = M.T which
    is exactly the lhsT the tensor engine wants."""
    y = np.eye(HD, dtype=np.float64)
    d = HD
    for _ in range(int(math.log2(d))):
        y = y.reshape(y.shape[:-1] + (2, -1))
        a, b = y[..., 0, :], y[..., 1, :]
        y = np.concatenate([a + b, a - b], axis=-1)
    scale = np.float32(d) ** np.float32(-0.5)
    return (y * scale).astype(F32)


def _build_nc():
    nc = bacc.Bacc("TRN2", target_bir_lowering=False)
    f32 = mybir.dt.float32
    bf16 = mybir.dt.bfloat16

    xp_d = nc.dram_tensor("xp", [NH * NG * 128, G * HALF], bf16, kind="ExternalInput")
    wp_d = nc.dram_tensor("wp", [NG * 128, G * HALF], bf16, kind="ExternalInput")
    hx_d = nc.dram_tensor("hx", [128, KC * RATIO], bf16, kind="ExternalInput")
    aug_d = nc.dram_tensor("aug", [RATIO, 1024], bf16, kind="ExternalInput")
    cpk_d = nc.dram_tensor("cpk", [128, C_TOT], f32, kind="ExternalInput")
    out_d = nc.dram_tensor("out", [128, NBC], f32, kind="ExternalOutput")

    AX = mybir.AxisListType
    OP = mybir.AluOpType
    AF = mybir.ActivationFunctionType

    with tile.TileContext(nc) as tc:
        with (
            tc.tile_pool(name="wts", bufs=1) as wts,
            tc.tile_pool(name="csts", bufs=1) as csts,
            tc.tile_pool(name="xs", bufs=4) as xs,
            tc.tile_pool(name="epi", bufs=2) as epi,
            tc.tile_pool(name="ps", bufs=1, space="PSUM") as ps,
            tc.tile_pool(name="mps", bufs=1, space="PSUM") as mps,
        ):
            # one PSUM bank per epilogue sub-chain for the small matmul
            # outputs (+ halo in misc[0]); avoids same-bank serialization:
            # [0:64 bcast][128:192 perm][256:320 fwht][384:448 varsum][448:456 halo]
            miscs = [
                mps.tile([128, HALF], mybir.dt.float32, name=f"misc{i}", tag=f"misc{i}")
                for i in range(NH * NSUB)
            ]
            # ---- const loads (SWDGE, off the x/w queue) ----
            hxsb = csts.tile([128, KC * RATIO], bf16, name="hxsb", tag="hxsb")
            nc.gpsimd.dma_start(out=hxsb, in_=hx_d[:, :])
            aug = csts.tile([RATIO, 1024], bf16, name="aug", tag="aug")
            nc.gpsimd.dma_start(out=aug, in_=aug_d[:, :])
            cpk = csts.tile([128, C_TOT], f32, name="cpk", tag="cpk")
            nc.gpsimd.dma_start(out=cpk, in_=cpk_d[:, :])
            waug = aug[:, 0:512]
            xaug = aug[:, 512:1024]
            cdup = cpk[:, C_CD:C_CD + NBC]
            sdup = cpk[:, C_SD:C_SD + NBC]
            pmat = cpk[:, C_PM:C_PM + 128]
            hmat = cpk[:, C_HM:C_HM + 128]
            normw = cpk[:, C_NW:C_NW + 1]
            onesk = cpk[:, C_OK:C_OK + 1]
            hbias = cpk[:, C_HB:C_HB + 1]
            row1 = cpk[0:1, C_R1:C_R1 + 128]
            epsap = cpk[0:1, C_EPS:C_EPS + 1]

            halo_sb = csts.tile([128, 2 * RATIO], f32, name="halo_sb", tag="halo_sb")
            carry = csts.tile([128, 2 * RATIO], f32, name="carry", tag="carry")
            outsb = csts.tile([128, NBC], f32, name="outsb", tag="outsb")

            # PE warm-up: dummy matmuls on a zeroed tile fill the initial DMA
            # wait so the HAM clock gate is at 8/8 when the real stream starts.
            zt = csts.tile([128, HALF], bf16, name="zt", tag="zt")
            nc.vector.memset(zt, 0.0)
            for i in range(8):
                nc.tensor.matmul(miscs[-1][:, :], zt[:, 0:128], zt[:, :],
                                 start=True, stop=True)

            # ---- interleaved weight/x loads on the sync HWDGE queue ----
            # first group's tiles split in half so matmuls start sooner
            # (Tile tracks subtile deps).
            wt = []
            xg_tiles = {}
            HB = G * HALF // 2
            for g in range(NG):
                wtg = wts.tile([128, G * HALF], bf16, name=f"wt{g}", tag=f"wt{g}")
                xg = xs.tile([128, G * HALF], bf16, name=f"xg0{g}", tag="xg")
                row = 128 * g
                if g == 0:
                    nc.sync.dma_start(out=xg[:, 0:HB], in_=xp_d[row:row + 128, 0:HB])
                    nc.sync.dma_start(out=wtg[:, 0:HB], in_=wp_d[row:row + 128, 0:HB])
                    nc.sync.dma_start(out=xg[:, HB:], in_=xp_d[row:row + 128, HB:])
                    nc.sync.dma_start(out=wtg[:, HB:], in_=wp_d[row:row + 128, HB:])
                else:
                    nc.sync.dma_start(out=wtg, in_=wp_d[row:row + 128, :])
                    nc.sync.dma_start(out=xg, in_=xp_d[row:row + 128, :])
                wt.append(wtg)
                xg_tiles[(0, g)] = xg

            def main_matmuls(h, psums, halo_psum):
                for g in range(NG):
                    if h == 0:
                        xg = xg_tiles.pop((0, g))
                    else:
                        xg = xs.tile([128, G * HALF], bf16, name=f"xg1{g}", tag="xg")
                        row = (h * NG + g) * 128
                        nc.sync.dma_start(out=xg, in_=xp_d[row:row + 128, :])
                    for cc in range(G):
                        first = g == 0 and cc == 0
                        for m in range(4):
                            nc.tensor.matmul(
                                psums[m],
                                wt[g][:, cc * HALF + 128 * m:cc * HALF + 128 * (m + 1)],
                                xg[:, cc * HALF:(cc + 1) * HALF],
                                start=first,
                                stop=False,
                            )
                    if h == 0:
                        # halo mini-matmuls ride along with the k-group whose
                        # weights are resident; keeps PE activity dense.
                        # NOTE: start=True clears has_written for the WHOLE
                        # bank, so only the very first matmul of the bank may
                        # set it — both m-groups then accumulate cleanly.
                        for cc in range(G):
                            c = g * G + cc
                            for m in range(2):
                                nc.tensor.matmul(
                                    halo_psum[:, 4 * m:4 * (m + 1)],
                                    wt[g][:, cc * HALF + 128 * m:cc * HALF + 128 * (m + 1)],
                                    hxsb[:, 4 * c:4 * (c + 1)],
                                    start=(c == 0 and m == 0),
                                    stop=False,
                                    skip_group_check=True,
                                )
                for m in range(4):
                    nc.tensor.matmul(
                        psums[m],
                        waug[:, 128 * m:128 * (m + 1)],
                        xaug[:, :],
                        start=False,
                        stop=True,
                    )
                if h == 0:
                    for m in range(2):
                        nc.tensor.matmul(
                            halo_psum[:, 4 * m:4 * (m + 1)],
                            waug[:, 128 * m:128 * (m + 1)],
                            xaug[:, 0:RATIO],
                            start=False,
                            stop=(m == 1),
                            skip_group_check=True,
                        )
                    nc.scalar.copy(out=halo_sb, in_=halo_psum[:, 0:2 * RATIO])

            def epilogue_sub(h, s, psums, E, M):
                """Softmax+RMS+rotary+FWHT for blocks [64s, 64s+64) of half h."""
                kv1p, sc1p, kv2p, sc2p = psums
                misc = miscs[h * NSUB + s]
                prev = halo_sb if h == 0 else carry
                c0 = 256 * s                      # E/M column base (E1 part)
                nq = NLOC * RATIO                 # 256 token-cols per sub
                if s == 0:
                    nc.scalar.activation(
                        E[:, 0:4], prev[:, RATIO:2 * RATIO], AF.Exp,
                        bias=(hbias if h == 0 else 0.0),
                    )
                    nc.scalar.activation(E[:, 4:nq], sc1p[:, 0:nq - 4], AF.Exp)
                    nc.vector.tensor_mul(M[:, 0:4], E[:, 0:4], prev[:, 0:RATIO])
                    nc.vector.tensor_mul(M[:, 4:nq], E[:, 4:nq], kv1p[:, 0:nq - 4])
                else:
                    nc.scalar.activation(E[:, c0:c0 + nq], sc1p[:, c0 - 4:c0 + nq - 4],
                                         AF.Exp)
                    nc.vector.tensor_mul(M[:, c0:c0 + nq], E[:, c0:c0 + nq],
                                         kv1p[:, c0 - 4:c0 + nq - 4])
                nc.scalar.activation(E[:, 512 + c0:512 + c0 + nq],
                                     sc2p[:, c0:c0 + nq], AF.Exp)
                nc.vector.tensor_mul(M[:, 512 + c0:512 + c0 + nq],
                                     E[:, 512 + c0:512 + c0 + nq],
                                     kv2p[:, c0:c0 + nq])

                Er = E.rearrange("p (t n q) -> p t n q", t=2, q=RATIO)
                Mr = M.rearrange("p (t n q) -> p t n q", t=2, q=RATIO)
                n0 = NLOC * s
                Z = epi.tile([128, NLOC], mybir.dt.float32, name=f"Z{h}{s}", tag="Z")
                nc.vector.tensor_reduce(
                    Z, Er[:, :, n0:n0 + NLOC, :].rearrange("p t n q -> p n t q"),
                    axis=AX.XY, op=OP.add)
                A = epi.tile([128, NLOC], mybir.dt.float32, name=f"A{h}{s}", tag="A")
                nc.vector.tensor_reduce(
                    A, Mr[:, :, n0:n0 + NLOC, :].rearrange("p t n q -> p n t q"),
                    axis=AX.XY, op=OP.add)
                Zr = epi.tile([128, NLOC], mybir.dt.float32, name=f"Zr{h}{s}", tag="Zr")
                nc.vector.reciprocal(Zr, Z)
                comp = epi.tile([128, NLOC], mybir.dt.float32, name=f"cp{h}{s}", tag="cp")
                nc.vector.tensor_mul(comp, A, Zr)
                sq = epi.tile([128, NLOC], mybir.dt.float32, name=f"sq{h}{s}", tag="sq")
                nc.scalar.square(sq, comp)

                nc.tensor.matmul(misc[0:1, 384:384 + NLOC], onesk, sq[:, :],
                                 start=True, stop=True)
                sd = epi.tile([1, NLOC], mybir.dt.float32, name=f"sd{h}{s}", tag="sd")
                nc.scalar.activation(sd, misc[0:1, 384:384 + NLOC], AF.Sqrt,
                                     bias=epsap)
                rs = epi.tile([1, NLOC], mybir.dt.float32, name=f"rs{h}{s}", tag="rs")
                nc.vector.reciprocal(rs, sd)
                nc.tensor.matmul(misc[:, 0:NLOC], row1, rs[:, :],
                                 start=True, stop=True)
                compn = epi.tile([128, NLOC], mybir.dt.float32,
                                 name=f"cn{h}{s}", tag="cn")
                nc.vector.scalar_tensor_tensor(
                    out=compn, in0=comp, scalar=normw, in1=misc[:, 0:NLOC],
                    op0=OP.mult, op1=OP.mult)
                nc.tensor.matmul(misc[:, 128:128 + NLOC], pmat, compn[:, :],
                                 start=True, stop=True)
                b0 = h * HALF // RATIO + n0       # global block base in this core
                t1 = epi.tile([128, NLOC], mybir.dt.float32, name=f"t1{h}{s}", tag="t1")
                nc.vector.tensor_mul(t1, misc[:, 128:128 + NLOC],
                                     sdup[:, b0:b0 + NLOC])
                t2 = epi.tile([128, NLOC], mybir.dt.float32, name=f"t2{h}{s}", tag="t2")
                nc.vector.tensor_mul(t2, compn, cdup[:, b0:b0 + NLOC])
                rot = epi.tile([128, NLOC], mybir.dt.float32, name=f"rt{h}{s}", tag="rt")
                nc.vector.tensor_add(rot, t1, t2)
                nc.tensor.matmul(misc[:, 256:256 + NLOC], hmat, rot[:, :],
                                 start=True, stop=True)
                nc.scalar.copy(out=outsb[:, b0:b0 + NLOC],
                               in_=misc[:, 256:256 + NLOC])

            for h in range(NH):
                psums = []
                for m, nm in enumerate(("kv1p", "sc1p", "kv2p", "sc2p")):
                    p = ps.tile([128, HALF], mybir.dt.float32, name=f"{nm}_{h}",
                                tag=nm)
                    psums.append(p)
                halo_psum = miscs[0][:, 448:456] if h == 0 else None
                main_matmuls(h, psums, halo_psum)
                if h == 0:
                    # carry last 4 token-cols of kv_lo/sc_lo into half 1
                    nc.scalar.copy(out=carry[:, 0:RATIO], in_=psums[0][:, HALF - 4:HALF])
                    nc.scalar.copy(out=carry[:, RATIO:2 * RATIO],
                                   in_=psums[1][:, HALF - 4:HALF])
                E = epi.tile([128, 2 * HALF], mybir.dt.float32, name=f"E{h}", tag="E")
                M = epi.tile([128, 2 * HALF], mybir.dt.float32, name=f"M{h}", tag="M")
                for s in range(NSUB):
                    epilogue_sub(h, s, psums, E, M)

            nc.sync.dma_start(out=out_d[:, :], in_=outsb)

    nc.finalize()
    return nc


def _prep_inputs(x, ape, wkv_w, wgate_w, norm_w, cos, sin):
    """Host-side packing of per-core input maps."""
    x = np.asarray(x, dtype=F32)[0]          # [SEQ, DIM]
    ape = np.asarray(ape, dtype=F32)         # [RATIO, 256]
    wkv_w = np.asarray(wkv_w, dtype=F32)     # [256, DIM]
    wgate_w = np.asarray(wgate_w, dtype=F32)
    norm_w = np.asarray(norm_w, dtype=F32)   # [HD]
    cos = np.asarray(cos, dtype=F32)         # [NB, 32]
    sin = np.asarray(sin, dtype=F32)

    xb = x.astype(BF16)

    w_comb = np.concatenate(
        [wkv_w[0:128], wgate_w[0:128], wkv_w[128:256], wgate_w[128:256]], axis=0
    )  # [512, DIM]
    wp = (
        w_comb.T.reshape(NG, G, 128, 512)
        .transpose(0, 2, 1, 3)
        .reshape(NG * 128, G * HALF)
        .astype(BF16)
    )
    wp = np.ascontiguousarray(wp)

    # aug: [waug | xaug] as one [4, 1024] bf16 tensor
    aug = np.zeros((RATIO, 1024), dtype=F32)
    aug[:, 0:128] = ape[:, 0:128]
    aug[:, 256:384] = ape[:, 128:256]
    for p in range(RATIO):
        aug[p, 512 + p::RATIO] = 1.0
    aug = aug.astype(BF16)

    pmat = np.zeros((128, 128), dtype=F32)
    for i in range(32):
        pmat[2 * i, 2 * i + 1] = 1.0
        pmat[2 * i + 1, 2 * i] = 1.0
    hmat = _fwht_mat()

    in_maps = []
    for c in range(NCORES):
        t0 = c * TOK
        seg = xb[t0:t0 + TOK]                       # [1024, DIM]
        arr = np.ascontiguousarray(seg.T)           # [DIM, 1024]
        arr = arr.reshape(KC, 128, NH, HALF)        # [c, p, h, t]
        xp = (
            arr.transpose(2, 0, 1, 3)               # [h, c, p, t]
            .reshape(NH, NG, G, 128, HALF)
            .transpose(0, 1, 3, 2, 4)               # [h, g, p, cc, t]
            .reshape(NH * NG * 128, G * HALF)
        )
        xp = np.ascontiguousarray(xp)

        if c == 0:
            halo = np.zeros((RATIO, DIM), dtype=BF16)
        else:
            halo = xb[t0 - RATIO:t0]
        hx = np.ascontiguousarray(
            halo.T.reshape(KC, 128, RATIO).transpose(1, 0, 2).reshape(128, KC * RATIO)
        )

        b0 = c * NBC
        cs = cos[b0:b0 + NBC]                       # [NBC, 32]
        ss = sin[b0:b0 + NBC]
        cpk = np.zeros((128, C_TOT), dtype=F32)
        cd = np.ones((128, NBC), dtype=F32)
        sd = np.zeros((128, NBC), dtype=F32)
        cd[0:64:2] = cs.T
        cd[1:64:2] = cs.T
        sd[0:64:2] = -ss.T
        sd[1:64:2] = ss.T
        cpk[:, C_CD:C_CD + NBC] = cd
        cpk[:, C_SD:C_SD + NBC] = sd
        cpk[:, C_PM:C_PM + 128] = pmat
        cpk[:, C_HM:C_HM + 128] = hmat
        cpk[:, C_NW] = norm_w
        cpk[:, C_OK] = 1.0 / HD
        cpk[:, C_HB] = NEGB if c == 0 else 0.0
        cpk[:, C_R1:C_R1 + 128] = 1.0
        cpk[0, C_EPS] = EPS

        in_maps.append(dict(xp=xp, wp=wp, hx=hx, aug=aug,
                            cpk=np.ascontiguousarray(cpk)))
    return in_maps


LAST_RESULTS = None


def kernel(x, ape, wkv_w, wgate_w, norm_w, cos, sin, start_pos=0,
           compress_state=None, **_unused):
    global LAST_RESULTS
    in_maps = _prep_inputs(x, ape, wkv_w, wgate_w, norm_w, cos, sin)
    if "nc" not in _cache:
        _cache["nc"] = _build_nc()
    nc = _cache["nc"]
    trace = bool(int(os.environ.get("KERNEL_TRACE", "0") or 0))
    tdir = os.environ.get("KERNEL_TRACE_DIR") or None
    res = run_bass_kernel_spmd(
        nc, in_maps, core_ids=list(range(NCORES)),
        trace=trace,
        trace_cores=[0] if trace else None,
        tmpdir=tdir,
    )
    LAST_RESULTS = res
    out = np.empty((1, NB, HD), dtype=F32)
    for c in range(NCORES):
        out[0, c * NBC:(c + 1) * NBC, :] = res.results[c]["out"].T
    return out


# revision 17
# speedup vs baseline: 1.0349x; 1.0104x over previous
"""Trainium2 Bass kernel for the DeepSeek-V4 indexer compressor (prefill).

Contract: kernel(**inputs) takes the FULL unsharded inputs (numpy) and
returns the FULL [1, 2048, 128] float32 output.

Strategy (8 NeuronCores, sequence-parallel):
  - Each core handles 1024 tokens = 256 compress blocks, plus a 4-token
    halo from the previous core (redundantly computed via tiny N=4
    matmuls interleaved into the first quarter's stream) for the overlap
    transform.
  - On-device layout is channel-major: the contraction dim (7168) lives
    on SBUF partitions, tokens on the free axis.  Host pre-transposes
    and bf16-casts x; wkv/wgate are fused into one [7168, 512] matrix
    with channel order [kv_lo | sc_lo | kv_hi | sc_hi] and the
    per-phase position embedding (ape) folded in as 4 extra contraction
    rows fed by a 0/1 phase-indicator rhs.
  - The token range is processed in 4 double-buffered quarters of 256
    tokens.  Each quarter accumulates 57 k-chunks into 2 packed PSUM
    banks (bank A = [kv_lo | sc_lo], bank B = [kv_hi | sc_hi]; only the
    first matmul per bank sets start=True since start clears has_written
    for the whole bank).  The epilogue chain per quarter (softmax over
    the 8 overlap rows via ACT exp + DVE quad-reduces, RMSNorm via
    ones-matmul + broadcast-matmul, rotary via a pair-swap permutation
    matmul, FWHT as one 128x128 matmul) gets a dedicated PSUM bank for
    its small matmul outputs and overlaps the next quarter's matmuls.
  - A few warm-up matmuls on a zeroed tile bridge the initial DMA wait
    so the PE clock gate is at 8/8 when the real stream starts.
  Output stays channel-major; host transposes back.
"""

import math
import os

import numpy as np
import ml_dtypes

import concourse.bass as bass
import concourse.bacc as bacc
import concourse.tile as tile
import concourse.mybir as mybir
from concourse.bass_utils import run_bass_kernel_spmd

BF16 = ml_dtypes.bfloat16
F32 = np.float32

# Problem dims (hardcoded per contract)
DIM = 7168
HD = 128
RATIO = 4
COFF = 2
SEQ = 8192
NB = SEQ // RATIO            # 2048 compressed blocks
NCORES = 8
TOK = SEQ // NCORES          # 1024 own tokens per core
NBC = TOK // RATIO           # 256 blocks per core
KC = DIM // 128              # 56 contraction chunks
G = 8                        # k-chunks per x/w DMA group
NG = KC // G                 # 7 groups
HALF = 512                   # tokens per x-DMA group column span
NH = 2                       # halves (x tile granularity)
NQ = 4                       # matmul/epilogue quarters
QT = TOK // NQ               # 256 tokens per quarter
NLOC = QT // RATIO           # 64 blocks per quarter
EPS = 1e-6
NEGB = -300.0                # exp(x - 300) == 0.0 in fp32 for masked rows
NDUMMY = 12                  # warm-up matmuls

# f32 const pack column layout
C_CD = 0            # cdup [128, 256]
C_SD = 256          # sdup [128, 256]
C_PM = 512          # pmat [128, 128]
C_HM = 640          # hmat [128, 128]
C_NW = 768          # norm_w [128, 1]
C_OK = 769          # ones/HD [128, 1]
C_HB = 770          # halo bias [128, 1]
C_R1 = 771          # all-ones block [128, 128] (row 0 used as [1,128])
C_EPS = 899         # eps (at [0:1, 899])
C_TOT = 900

_cache = {}


def _fwht_mat():
    """fwht(v) = M @ v for the reference's butterfly; fwht(I) = M.T which
    is exactly the lhsT the tensor engine wants."""
    y = np.eye(HD, dtype=np.float64)
    d = HD
    for _ in range(int(math.log2(d))):
        y = y.reshape(y.shape[:-1] + (2, -1))
        a, b = y[..., 0, :], y[..., 1, :]
        y = np.concatenate([a + b, a - b], axis=-1)
    scale = np.float32(d) ** np.float32(-0.5)
    return (y * scale).astype(F32)


def _build_nc():
    nc = bacc.Bacc("TRN2", target_bir_lowering=False)
    f32 = mybir.dt.float32
    bf16 = mybir.dt.bfloat16

    xp_d = nc.dram_tensor("xp", [NH * NG * 128, G * HALF], bf16, kind="ExternalInput")
    wp_d = nc.dram_tensor("wp", [NG * 128, G * HALF], bf16, kind="ExternalInput")
    hx_d = nc.dram_tensor("hx", [128, KC * RATIO], bf16, kind="ExternalInput")
    aug_d = nc.dram_tensor("aug", [RATIO, 1024], bf16, kind="ExternalInput")
    cpk_d = nc.dram_tensor("cpk", [128, C_TOT], f32, kind="ExternalInput")
    out_d = nc.dram_tensor("out", [128, NBC], f32, kind="ExternalOutput")

    AX = mybir.AxisListType
    OP = mybir.AluOpType
    AF = mybir.ActivationFunctionType

    with tile.TileContext(nc) as tc:
        with (
            tc.tile_pool(name="wts", bufs=1) as wts,
            tc.tile_pool(name="csts", bufs=1) as csts,
            tc.tile_pool(name="xs", bufs=8) as xs,
            tc.tile_pool(name="epi", bufs=2) as epi,
            tc.tile_pool(name="ps", bufs=2, space="PSUM") as ps,
            tc.tile_pool(name="mps", bufs=1, space="PSUM") as mps,
        ):
            # one PSUM bank per quarter for the epilogue's small matmul
            # outputs (+ halo in misc[0]); avoids same-bank serialization:
            # [0:64 bcast][128:192 perm][256:320 fwht][384:448 varsum][448:456 halo]
            miscs = [
                mps.tile([128, HALF], mybir.dt.float32, name=f"misc{i}", tag=f"misc{i}")
                for i in range(NQ)
            ]

            hxsb = csts.tile([128, KC * RATIO], bf16, name="hxsb", tag="hxsb")
            nc.gpsimd.dma_start(out=hxsb, in_=hx_d[:, :])
            aug = csts.tile([RATIO, 1024], bf16, name="aug", tag="aug")
            nc.gpsimd.dma_start(out=aug, in_=aug_d[:, :])
            cpk = csts.tile([128, C_TOT], f32, name="cpk", tag="cpk")
            nc.gpsimd.dma_start(out=cpk, in_=cpk_d[:, :])
            waug = aug[:, 0:512]
            xaug = aug[:, 512:1024]
            cdup = cpk[:, C_CD:C_CD + NBC]
            sdup = cpk[:, C_SD:C_SD + NBC]
            pmat = cpk[:, C_PM:C_PM + 128]
            hmat = cpk[:, C_HM:C_HM + 128]
            normw = cpk[:, C_NW:C_NW + 1]
            onesk = cpk[:, C_OK:C_OK + 1]
            hbias = cpk[:, C_HB:C_HB + 1]
            row1 = cpk[0:1, C_R1:C_R1 + 128]
            epsap = cpk[0:1, C_EPS:C_EPS + 1]

            halo_sb = csts.tile([128, 2 * RATIO], f32, name="halo_sb", tag="halo_sb")
            outsb = csts.tile([128, NBC], f32, name="outsb", tag="outsb")

            # PE warm-up: dummy matmuls on a zeroed tile bridge the initial
            # DMA wait so the HAM clock gate is at 8/8 for the real stream.
            zt = csts.tile([128, HALF], bf16, name="zt", tag="zt")
            nc.vector.memset(zt, 0.0)
            for i in range(NDUMMY):
                nc.tensor.matmul(miscs[-1][:, :], zt[:, 0:128], zt[:, :],
                                 start=True, stop=True)

            # ---- interleaved weight/x loads on the sync HWDGE queue ----
            # group 0 is split into 1024-col chunks so matmuls start sooner
            # (Tile tracks subtile deps).
            wt = []
            xg_tiles = {}
            for g in range(NG):
                wtg = wts.tile([128, G * HALF], bf16, name=f"wt{g}", tag=f"wt{g}")
                xg = xs.tile([128, G * HALF], bf16, name=f"xg0{g}", tag="xg")
                row = 128 * g
                if g == 0:
                    for a in range(0, G * HALF, 1024):
                        nc.sync.dma_start(out=xg[:, a:a + 1024],
                                          in_=xp_d[row:row + 128, a:a + 1024])
                        nc.sync.dma_start(out=wtg[:, a:a + 1024],
                                          in_=wp_d[row:row + 128, a:a + 1024])
                else:
                    nc.sync.dma_start(out=wtg, in_=wp_d[row:row + 128, :])
                    nc.sync.dma_start(out=xg, in_=xp_d[row:row + 128, :])
                wt.append(wtg)
                xg_tiles[(0, g)] = xg

            def quarter_matmuls(q, bankA, bankB):
                """57-chunk accumulation for 256 tokens into 2 packed banks.

                start=True clears has_written for the WHOLE bank, so only
                the first matmul per bank sets it; the second m-group in
                each bank then accumulates cleanly (guarded by PE program
                order).
                """
                h, o = divmod(q, 2)
                o *= QT
                outs = (bankA[:, 0:QT], bankA[:, QT:2 * QT],
                        bankB[:, 0:QT], bankB[:, QT:2 * QT])
                for g in range(NG):
                    if (h, g) in xg_tiles:
                        xg = xg_tiles[(h, g)]
                    else:
                        xg = xs.tile([128, G * HALF], bf16, name=f"xg1{g}", tag="xg")
                        row = (h * NG + g) * 128
                        nc.sync.dma_start(out=xg, in_=xp_d[row:row + 128, :])
                        xg_tiles[(h, g)] = xg
                    for cc in range(G):
                        first = g == 0 and cc == 0
                        for m in range(4):
                            nc.tensor.matmul(
                                outs[m],
                                wt[g][:, cc * HALF + 128 * m:cc * HALF + 128 * (m + 1)],
                                xg[:, cc * HALF + o:cc * HALF + o + QT],
                                start=first and m % 2 == 0,
                                stop=False,
                                skip_group_check=True,
                            )
                    if q == 0:
                        # halo mini-matmuls ride along with the k-group whose
                        # weights are resident; keeps PE activity dense.
                        hp = miscs[0][:, 448:456]
                        for cc in range(G):
                            c = g * G + cc
                            for m in range(2):
                                nc.tensor.matmul(
                                    hp[:, 4 * m:4 * (m + 1)],
                                    wt[g][:, cc * HALF + 128 * m:cc * HALF + 128 * (m + 1)],
                                    hxsb[:, 4 * c:4 * (c + 1)],
                                    start=(c == 0 and m == 0),
                                    stop=False,
                                    skip_group_check=True,
                                )
                for m in range(4):
                    nc.tensor.matmul(
                        outs[m],
                        waug[:, 128 * m:128 * (m + 1)],
                        xaug[:, o:o + QT],
                        start=False,
                        stop=m % 2 == 1,
                        skip_group_check=True,
                    )
                if q == 0:
                    hp = miscs[0][:, 448:456]
                    for m in range(2):
                        nc.tensor.matmul(
                            hp[:, 4 * m:4 * (m + 1)],
                            waug[:, 128 * m:128 * (m + 1)],
                            xaug[:, 0:RATIO],
                            start=False,
                            stop=(m == 1),
                            skip_group_check=True,
                        )
                    nc.scalar.copy(out=halo_sb, in_=hp)

            def epilogue(q, bankA, bankB, prev, carry):
                """Softmax+RMS+rotary+FWHT for the 64 blocks of quarter q."""
                kv1p = bankA[:, 0:QT]
                sc1p = bankA[:, QT:2 * QT]
                kv2p = bankB[:, 0:QT]
                sc2p = bankB[:, QT:2 * QT]
                misc = miscs[q]
                if carry is not None:
                    nc.scalar.copy(out=carry[:, 0:RATIO], in_=kv1p[:, QT - 4:QT])
                    nc.scalar.copy(out=carry[:, RATIO:2 * RATIO],
                                   in_=sc1p[:, QT - 4:QT])

                E = epi.tile([128, 2 * QT], mybir.dt.float32, name=f"E{q}", tag="E")
                M = epi.tile([128, 2 * QT], mybir.dt.float32, name=f"M{q}", tag="M")
                nc.scalar.activation(
                    E[:, 0:4], prev[:, RATIO:2 * RATIO], AF.Exp,
                    bias=(hbias if q == 0 else 0.0),
                )
                nc.scalar.activation(E[:, 4:QT], sc1p[:, 0:QT - 4], AF.Exp)
                nc.scalar.activation(E[:, QT:2 * QT], sc2p[:, :], AF.Exp)
                nc.vector.tensor_mul(M[:, 0:4], E[:, 0:4], prev[:, 0:RATIO])
                nc.vector.tensor_mul(M[:, 4:QT], E[:, 4:QT], kv1p[:, 0:QT - 4])
                nc.vector.tensor_mul(M[:, QT:2 * QT], E[:, QT:2 * QT], kv2p[:, :])

                Z = epi.tile([128, NLOC], mybir.dt.float32, name=f"Z{q}", tag="Z")
                nc.vector.tensor_reduce(
                    Z, E.rearrange("p (t n q) -> p n t q", t=2, q=RATIO),
                    axis=AX.XY, op=OP.add)
                A = epi.tile([128, NLOC], mybir.dt.float32, name=f"A{q}", tag="A")
                nc.vector.tensor_reduce(
                    A, M.rearrange("p (t n q) -> p n t q", t=2, q=RATIO),
                    axis=AX.XY, op=OP.add)
                Zr = epi.tile([128, NLOC], mybir.dt.float32, name=f"Zr{q}", tag="Zr")
                nc.vector.reciprocal(Zr, Z)
                comp = epi.tile([128, NLOC], mybir.dt.float32, name=f"cp{q}", tag="cp")
                nc.vector.tensor_mul(comp, A, Zr)
                sq = epi.tile([128, NLOC], mybir.dt.float32, name=f"sq{q}", tag="sq")
                nc.scalar.square(sq, comp)

                nc.tensor.matmul(misc[0:1, 384:384 + NLOC], onesk, sq[:, :],
                                 start=True, stop=True)
                sd = epi.tile([1, NLOC], mybir.dt.float32, name=f"sd{q}", tag="sd")
                nc.scalar.activation(sd, misc[0:1, 384:384 + NLOC], AF.Sqrt,
                                     bias=epsap)
                rs = epi.tile([1, NLOC], mybir.dt.float32, name=f"rs{q}", tag="rs")
                nc.vector.reciprocal(rs, sd)
                nc.tensor.matmul(misc[:, 0:NLOC], row1, rs[:, :],
                                 start=True, stop=True)
                compn = epi.tile([128, NLOC], mybir.dt.float32, name=f"cn{q}", tag="cn")
                nc.vector.scalar_tensor_tensor(
                    out=compn, in0=comp, scalar=normw, in1=misc[:, 0:NLOC],
                    op0=OP.mult, op1=OP.mult)
                nc.tensor.matmul(misc[:, 128:128 + NLOC], pmat, compn[:, :],
                                 start=True, stop=True)
                b0 = q * NLOC
                t1 = epi.tile([128, NLOC], mybir.dt.float32, name=f"t1{q}", tag="t1")
                nc.vector.tensor_mul(t1, misc[:, 128:128 + NLOC],
                                     sdup[:, b0:b0 + NLOC])
                t2 = epi.tile([128, NLOC], mybir.dt.float32, name=f"t2{q}", tag="t2")
                nc.vector.tensor_mul(t2, compn, cdup[:, b0:b0 + NLOC])
                rot = epi.tile([128, NLOC], mybir.dt.float32, name=f"rt{q}", tag="rt")
                nc.vector.tensor_add(rot, t1, t2)
                nc.tensor.matmul(misc[:, 256:256 + NLOC], hmat, rot[:, :],
                                 start=True, stop=True)
                nc.scalar.copy(out=outsb[:, b0:b0 + NLOC],
                               in_=misc[:, 256:256 + NLOC])

            prev = halo_sb
            for q in range(NQ):
                bankA = ps.tile([128, 2 * QT], mybir.dt.float32,
                                name=f"bankA{q}", tag="bankA")
                bankB = ps.tile([128, 2 * QT], mybir.dt.float32,
                                name=f"bankB{q}", tag="bankB")
                quarter_matmuls(q, bankA, bankB)
                carry = None
                if q < NQ - 1:
                    carry = csts.tile([128, 2 * RATIO], f32, name=f"carry{q}",
                                      tag=f"carry{q}")
                epilogue(q, bankA, bankB, prev, carry)
                prev = carry

            nc.sync.dma_start(out=out_d[:, :], in_=outsb)

    nc.finalize()
    return nc


def _prep_inputs(x, ape, wkv_w, wgate_w, norm_w, cos, sin):
    """Host-side packing of per-core input maps."""
    x = np.asarray(x, dtype=F32)[0]          # [SEQ, DIM]
    ape = np.asarray(ape, dtype=F32)         # [RATIO, 256]
    wkv_w = np.asarray(wkv_w, dtype=F32)     # [256, DIM]
    wgate_w = np.asarray(wgate_w, dtype=F32)
    norm_w = np.asarray(norm_w, dtype=F32)   # [HD]
    cos = np.asarray(cos, dtype=F32)         # [NB, 32]
    sin = np.asarray(sin, dtype=F32)

    xb = x.astype(BF16)

    w_comb = np.concatenate(
        [wkv_w[0:128], wgate_w[0:128], wkv_w[128:256], wgate_w[128:256]], axis=0
    )  # [512, DIM]
    wp = (
        w_comb.T.reshape(NG, G, 128, 512)
        .transpose(0, 2, 1, 3)
        .reshape(NG * 128, G * HALF)
        .astype(BF16)
    )
    wp = np.ascontiguousarray(wp)

    # aug: [waug | xaug] as one [4, 1024] bf16 tensor
    aug = np.zeros((RATIO, 1024), dtype=F32)
    aug[:, 0:128] = ape[:, 0:128]
    aug[:, 256:384] = ape[:, 128:256]
    for p in range(RATIO):
        aug[p, 512 + p::RATIO] = 1.0
    aug = aug.astype(BF16)

    pmat = np.zeros((128, 128), dtype=F32)
    for i in range(32):
        pmat[2 * i, 2 * i + 1] = 1.0
        pmat[2 * i + 1, 2 * i] = 1.0
    hmat = _fwht_mat()

    in_maps = []
    for c in range(NCORES):
        t0 = c * TOK
        seg = xb[t0:t0 + TOK]                       # [1024, DIM]
        arr = np.ascontiguousarray(seg.T)           # [DIM, 1024]
        arr = arr.reshape(KC, 128, NH, HALF)        # [c, p, h, t]
        xp = (
            arr.transpose(2, 0, 1, 3)               # [h, c, p, t]
            .reshape(NH, NG, G, 128, HALF)
            .transpose(0, 1, 3, 2, 4)               # [h, g, p, cc, t]
            .reshape(NH * NG * 128, G * HALF)
        )
        xp = np.ascontiguousarray(xp)

        if c == 0:
            halo = np.zeros((RATIO, DIM), dtype=BF16)
        else:
            halo = xb[t0 - RATIO:t0]
        hx = np.ascontiguousarray(
            halo.T.reshape(KC, 128, RATIO).transpose(1, 0, 2).reshape(128, KC * RATIO)
        )

        b0 = c * NBC
        cs = cos[b0:b0 + NBC]                       # [NBC, 32]
        ss = sin[b0:b0 + NBC]
        cpk = np.zeros((128, C_TOT), dtype=F32)
        cd = np.ones((128, NBC), dtype=F32)
        sd = np.zeros((128, NBC), dtype=F32)
        cd[0:64:2] = cs.T
        cd[1:64:2] = cs.T
        sd[0:64:2] = -ss.T
        sd[1:64:2] = ss.T
        cpk[:, C_CD:C_CD + NBC] = cd
        cpk[:, C_SD:C_SD + NBC] = sd
        cpk[:, C_PM:C_PM + 128] = pmat
        cpk[:, C_HM:C_HM + 128] = hmat
        cpk[:, C_NW] = norm_w
        cpk[:, C_OK] = 1.0 / HD
        cpk[:, C_HB] = NEGB if c == 0 else 0.0
        cpk[:, C_R1:C_R1 + 128] = 1.0
        cpk[0, C_EPS] = EPS

        in_maps.append(dict(xp=xp, wp=wp, hx=hx, aug=aug,
                            cpk=np.ascontiguousarray(cpk)))
    return in_maps


LAST_RESULTS = None


def kernel(x, ape, wkv_w, wgate_w, norm_w, cos, sin, start_pos=0,
           compress_state=None, **_unused):
    global LAST_RESULTS
    in_maps = _prep_inputs(x, ape, wkv_w, wgate_w, norm_w, cos, sin)
    if "nc" not in _cache:
        _cache["nc"] = _build_nc()
    nc = _cache["nc"]
    trace = bool(int(os.environ.get("KERNEL_TRACE", "0") or 0))
    tdir = os.environ.get("KERNEL_TRACE_DIR") or None
    res = run_bass_kernel_spmd(
        nc, in_maps, core_ids=list(range(NCORES)),
        trace=trace,
        trace_cores=[0] if trace else None,
        tmpdir=tdir,
    )
    LAST_RESULTS = res
    out = np.empty((1, NB, HD), dtype=F32)
    for c in range(NCORES):
        out[0, c * NBC:(c + 1) * NBC, :] = res.results[c]["out"].T
    return out
